# revision 49
# baseline (speedup 1.0000x reference)
"""Trainium2 Bass kernel: 8-expert top-2 MoE MLP (SwiGLU), 8 cores.

Hidden-dim-sliced expert parallelism, 4 phases: every expert's FFN is split
into four M/4=512 hidden slices (gate/up pairs stay together, so the split is
exact; the four mm2 partial outputs are summed on the host in the combine).
Experts are paired by adjacent load rank; phase i runs both experts of pair i
— cores 0-3 hold slices 0-3 of the first expert, cores 4-7 of the second —
with capacity C_i = max(pair_i loads).  Per-core work is sum(C_i)/4 ~ 2057
full-width token-equivalents vs 2150 under plain EP and 2079 under the
2-phase variant.  Weights per core stay 12.6 MB bf16.

Per phase, per core (one expert quarter-slice, MH4=512, KH4=4):
  H'^T[2*MH4, C] = (W13 slice stationary).T x X^T moving  (contract D)
  H^T = silu(gate) * up                                   (ACT + DVE)
  O^T[D, C]      = (W2 slice stationary) x H^T moving     (contract MH4)

x tiles share block tags across phases (bufs=2) so only ~2 phases of x are
SBUF-resident; phase i+2's x DMAs are emitted after phase i's compute so
their WAR waits are satisfied when the sync queue reaches them.

Schedule facts (trace-verified on ancestors): ~8.3us framework preamble;
8 warm-up matmuls fix the 1.2->2.4 GHz HAM throttle inside the DMA ramp;
matmuls issue at N/2.4+2.5 ns with LDWEIGHTS hidden; phase-0 block 0 is
chunk-gated k-granularly against the ~0.33 MB/us HBM stream; the kernel ends
on a 128-token block to shorten the final cast+DMA+teardown chain.  A host
spot-check of 128 tokens guards against the (rare, timing-dependent) device
corruption seen once on this rig; on mismatch the NEFF re-runs (<=2 retries).
"""

from contextlib import ExitStack

import ml_dtypes
import numpy as np

import concourse.bass as bass  # noqa: F401  (AP helpers)
import concourse.tile as tile
from concourse import bacc, mybir
from concourse.bass_utils import run_bass_kernel_spmd

# nn_MoEMLP_82617990905863 (hardcoded per contract)
B, S, D = 4, 2048, 1024
T = B * S               # 8192 tokens
E = 8                   # experts
TOPK = 2
M = 2048                # full MOE_DIM (w13 = [D, 2M], w2 = [M, D])
NP = 4                  # phases
MH4 = M // NP           # hidden slice width per core-phase = 512
TB = 512                # token block = moving free dim
KD = D // 128           # 8 contraction tiles for X @ W13
KH4 = MH4 // 128        # 4 contraction tiles for H @ W2 (per slice)
# phase-0 w13 chunk schedule over its 4 j-tiles: singles first
W13_CHUNKS = [[0], [1], [2, 3]]

_NC_CACHE: dict[tuple, object] = {}
last_results = None     # BassKernelResults of the most recent run (for test.py)


def _blocks(C: int, tail_split: bool = False) -> list[int]:
    sizes = []
    left = C
    while left > TB:
        sizes.append(TB)
        left -= TB
    if left:
        sizes.append(left)
    # tiny remainders run at the per-MM issue floor — merge them into the
    # previous block, then split so the kernel ends on a narrow block
    # (the final cast+DMA+teardown chain scales with last-block width)
    if len(sizes) >= 2 and sizes[-1] < 192:
        sizes[-2:] = [sizes[-2] + sizes[-1]]
    if tail_split and sizes[-1] > 192:
        cut = 256 if sizes[-1] > 640 else 128
        sizes[-1:] = [sizes[-1] - cut, cut]
    return sizes


def _build(Cs: tuple):
    """NP sequential expert-slice phases. Phase 0 has the chunk-gated DMA
    ramp; later phases' weights are fully prefetched."""
    dt = mybir.dt
    odt = dt.bfloat16
    nc = bacc.Bacc(
        "TRN2", target_bir_lowering=False, debug=False, enable_asserts=False
    )

    xt, w13, w2, ot = [], [], [], []
    for i, C in enumerate(Cs):
        xt.append(
            nc.dram_tensor(f"xt{i}", [128, KD * C], dt.bfloat16,
                           kind="ExternalInput").ap()
        )
        w13.append(
            nc.dram_tensor(f"w13{i}", [128, KD * 2 * MH4], dt.bfloat16,
                           kind="ExternalInput").ap()
        )
        w2.append(
            nc.dram_tensor(f"w2{i}", [128, KH4 * D], dt.bfloat16,
                           kind="ExternalInput").ap()
        )
        ot.append(
            nc.dram_tensor(f"ot{i}", [128, KD * C], odt,
                           kind="ExternalOutput").ap()
        )

    sizes = [_blocks(C, True) for C in Cs]
    offs = []
    for i in range(NP):
        o = [0]
        for s in sizes[i]:
            o.append(o[-1] + s)
        offs.append(o)

    with tile.TileContext(nc) as tc, ExitStack() as ctx:
        wpool = ctx.enter_context(tc.tile_pool(name="w", bufs=1))
        xpool = ctx.enter_context(tc.tile_pool(name="x", bufs=1))
        xring = ctx.enter_context(tc.tile_pool(name="xr", bufs=2))
        spool = ctx.enter_context(tc.tile_pool(name="s", bufs=4))
        hpool = ctx.enter_context(tc.tile_pool(name="h", bufs=2))
        opool = ctx.enter_context(tc.tile_pool(name="o", bufs=2))
        pg = ctx.enter_context(tc.tile_pool(name="pg", bufs=3, space="PSUM"))
        pu = ctx.enter_context(tc.tile_pool(name="pu", bufs=3, space="PSUM"))
        po = ctx.enter_context(tc.tile_pool(name="po", bufs=2, space="PSUM"))

        # --- PE clock warm-up during the DMA-bound ramp ---
        warm = xpool.tile([128, 512], dt.bfloat16, tag="warm")
        nc.vector.memset(warm[:], 0)
        wpsum = po.tile([128, 512], dt.float32, tag="po")
        for _ in range(8):
            nc.tensor.matmul(
                wpsum[:], warm[:, 0:128], warm[:, 0:512], start=True, stop=True
            )

        # --- phase-0 ramp: x block 0 and w13[0] chunk 0 as interleaved
        # k-pair tiles; w13[0] chunk 1 issued mid-stream ---
        x0k, w0q = [], []
        for q in range(4):
            tx = xpool.tile([128, 2 * TB], dt.bfloat16, tag=f"x0q{q}")
            nc.sync.dma_start(tx[:], xt[0][:, q * 2 * TB : (q + 1) * 2 * TB])
            x0k.append(tx)
            tw = wpool.tile([128, 512], dt.bfloat16, tag=f"w0q{q}")
            nc.sync.dma_start(tw[:], w13[0][:, q * 512 : (q + 1) * 512])
            w0q.append(tw)
            if q == 1:
                wc1 = wpool.tile([128, KD * 2 * 128], dt.bfloat16, tag="wc1")
                nc.sync.dma_start(
                    wc1[:], w13[0][:, KD * 256 : KD * 256 + KD * 2 * 128]
                )

        j_chunk = {}            # j -> (chunk idx, local jj, cgw)
        w13_offs = []
        off = 0
        for ci, js in enumerate(W13_CHUNKS):
            w13_offs.append(off)
            for jj, j in enumerate(js):
                j_chunk[j] = (ci, jj, 128 * len(js))
            off += KD * 2 * 128 * len(js)

        wt0 = [None, wc1]
        for ci in range(2, len(W13_CHUNKS)):
            cgw = 128 * len(W13_CHUNKS[ci])
            t = wpool.tile([128, KD * 2 * cgw], dt.bfloat16, tag=f"wc0{ci}")
            nc.sync.dma_start(
                t[:], w13[0][:, w13_offs[ci] : w13_offs[ci] + KD * 2 * cgw]
            )
            wt0.append(t)

        # x blocks share tags across phases via a bufs=2 ring; phase i+2's
        # loads are emitted after phase i's compute (WAR then satisfied by
        # the time the sync queue reaches them — no head-of-line jam)
        xb = [[None] * len(sizes[i]) for i in range(NP)]

        def load_x(i, b):
            n = sizes[i][b]
            t = xring.tile([128, KD * n], dt.bfloat16, tag=f"xb{b}", name="xb")
            xb[i][b] = t
            nc.sync.dma_start(
                t[:], xt[i][:, KD * offs[i][b] : KD * offs[i][b] + KD * n]
            )

        for b in range(1, len(sizes[0])):
            load_x(0, b)
        w2t, wtf = [None] * NP, [None] * NP
        w2t[0] = wpool.tile([128, KH4 * D], dt.bfloat16, tag="w2p0", name="w2t0")
        nc.sync.dma_start(w2t[0][:], w2[0][:, :])
        for i in range(1, NP):
            wtf[i] = wpool.tile([128, KD * 2 * MH4], dt.bfloat16, tag=f"wp{i}", name="wtf")
            nc.sync.dma_start(wtf[i][:], w13[i][:, :])
            w2t[i] = wpool.tile([128, KH4 * D], dt.bfloat16, tag=f"w2p{i}", name="w2t")
            nc.sync.dma_start(w2t[i][:], w2[i][:, :])
        for b in range(len(sizes[1])):
            load_x(1, b)

        def x_slice(i, b, k, n):
            if i == 0 and b == 0:
                q, kk = divmod(k, 2)
                return x0k[q][:, kk * n : (kk + 1) * n]
            return xb[i][b][:, k * n : (k + 1) * n]

        def w13_slice(i, j, k, gate):
            if i == 0:
                ci, jj, cgw = j_chunk[j]
                if ci == 0:
                    q, kk = divmod(k, 2)
                    base = kk * 256 + (0 if gate else 128)
                    return w0q[q][:, base : base + 128]
                base = k * 2 * cgw + (0 if gate else cgw) + jj * 128
                return wt0[ci][:, base : base + 128]
            base = k * 2 * MH4 + (0 if gate else MH4) + j * 128
            return wtf[i][:, base : base + 128]

        def emit_phase(i):
            for b in range(len(sizes[i])):
                n = sizes[i][b]
                h_t = []
                for j in range(KH4):
                    g = pg.tile([128, TB], dt.float32, tag="pg", name="g")[:, 0:n]
                    u = pu.tile([128, TB], dt.float32, tag="pu", name="u")[:, 0:n]
                    for k in range(KD):
                        nc.tensor.matmul(
                            g[:],
                            w13_slice(i, j, k, True),
                            x_slice(i, b, k, n),
                            start=(k == 0),
                            stop=(k == KD - 1),
                        )
                    for k in range(KD):
                        nc.tensor.matmul(
                            u[:],
                            w13_slice(i, j, k, False),
                            x_slice(i, b, k, n),
                            start=(k == 0),
                            stop=(k == KD - 1),
                        )
                    gs = spool.tile(
                        [128, TB], dt.float32, tag="gs", name="gs"
                    )[:, 0:n]
                    nc.scalar.activation(
                        gs[:], g[:], mybir.ActivationFunctionType.Silu
                    )
                    h = hpool.tile(
                        [128, TB], dt.bfloat16, tag=f"h{j}", name="h"
                    )[:, 0:n]
                    nc.vector.tensor_mul(h[:], gs[:], u[:])
                    h_t.append(h)
                ob = opool.tile(
                    [128, (KD - 1) * TB], odt, tag="o", name="ob"
                )[:, 0 : (KD - 1) * n]
                o7 = opool.tile([128, TB], odt, tag="o7", name="o7")[:, 0:n]
                for d in range(KD):
                    p = po.tile([128, TB], dt.float32, tag="po", name="p")[:, 0:n]
                    for j in range(KH4):
                        nc.tensor.matmul(
                            p[:],
                            w2t[i][:, j * D + d * 128 : j * D + (d + 1) * 128],
                            h_t[j][:],
                            start=(j == 0),
                            stop=(j == KH4 - 1),
                        )
                    if d < KD - 1:
                        nc.vector.tensor_copy(ob[:, d * n : (d + 1) * n], p[:])
                    else:
                        nc.vector.tensor_copy(o7[:], p[:])
                    if d == KD - 2:
                        nc.sync.dma_start(
                            ot[i][
                                :,
                                KD * offs[i][b] : KD * offs[i][b] + (KD - 1) * n,
                            ],
                            ob[:],
                        )
                nc.sync.dma_start(
                    ot[i][
                        :,
                        KD * offs[i][b] + (KD - 1) * n : KD * offs[i][b] + KD * n,
                    ],
                    o7[:],
                )

        for i in range(NP):
            emit_phase(i)
            if i + 2 < NP:
                for b in range(len(sizes[i + 2])):
                    load_x(i + 2, b)

    nc.compile()
    return nc


def _stage_x(xg: np.ndarray) -> np.ndarray:
    """[C, D] gathered tokens -> [128, block-major (b, k, tok)] bf16."""
    C = xg.shape[0]
    a = np.ascontiguousarray(xg.T).reshape(KD, 128, C)       # [k, p, tok]
    blocks = []
    c0 = 0
    for n in _blocks(C, True):
        blocks.append(a[:, :, c0 : c0 + n].transpose(1, 0, 2).reshape(128, KD * n))
        c0 += n
    return np.ascontiguousarray(np.concatenate(blocks, axis=1))


def _stage_w13_slice(w: np.ndarray, s: int, chunked: bool) -> np.ndarray:
    """[D, 2M] gate|up, hidden slice s of width MH4 -> [128, ...] bf16.
    chunked=True lays out per W13_CHUNKS (phase 0); else k-major full."""
    g = w[:, s * MH4 : (s + 1) * MH4]
    u = w[:, M + s * MH4 : M + (s + 1) * MH4]
    if not chunked:
        a = np.concatenate([g, u], axis=1)                    # [D, 2*MH4]
        return np.ascontiguousarray(
            a.reshape(KD, 128, 2 * MH4).transpose(1, 0, 2).reshape(
                128, KD * 2 * MH4
            )
        )
    parts = []
    for js in W13_CHUNKS:
        cgw = 128 * len(js)
        cols_g = np.concatenate([g[:, j * 128 : (j + 1) * 128] for j in js], axis=1)
        cols_u = np.concatenate([u[:, j * 128 : (j + 1) * 128] for j in js], axis=1)
        a = np.concatenate([cols_g, cols_u], axis=1)         # [D, 2cgw]
        parts.append(
            a.reshape(KD, 128, 2 * cgw).transpose(1, 0, 2).reshape(128, KD * 2 * cgw)
        )
    return np.ascontiguousarray(np.concatenate(parts, axis=1))


def _stage_w2_slice(w: np.ndarray, s: int) -> np.ndarray:
    """[M, D], hidden slice s -> [128, (k, d)] bf16."""
    ws = w[s * MH4 : (s + 1) * MH4, :]
    return np.ascontiguousarray(
        ws.reshape(KH4, 128, D).transpose(1, 0, 2).reshape(128, KH4 * D)
    )


def _unstage_o(ote: np.ndarray, C: int) -> np.ndarray:
    """[128, block-major (b, d, tok)] -> [D, C] (inverse of the x staging)."""
    blocks = []
    c0 = 0
    for n in _blocks(C, True):
        blk = ote[:, KD * c0 : KD * c0 + KD * n].reshape(128, KD, n)
        blocks.append(blk.transpose(1, 0, 2).reshape(D, n))
        c0 += n
    return np.concatenate(blocks, axis=1)


def _route(xf: np.ndarray, moe_router: np.ndarray):
    """Top-2 routing on host. Returns per-expert (rows, weights)."""
    logits = xf @ moe_router                      # [T, E] f32
    top1 = np.argmax(logits, axis=1)
    tmp = logits.copy()
    tmp[np.arange(T), top1] = -np.inf
    top2 = np.argmax(tmp, axis=1)
    l1 = logits[np.arange(T), top1]
    l2 = logits[np.arange(T), top2]
    mx = np.maximum(l1, l2)
    e1 = np.exp(l1 - mx)
    e2 = np.exp(l2 - mx)
    s = e1 + e2
    w1 = (e1 / s).astype(np.float32)
    w2 = (e2 / s).astype(np.float32)
    per_expert = []
    for e in range(E):
        r1 = np.where(top1 == e)[0]
        r2 = np.where(top2 == e)[0]
        rows = np.concatenate([r1, r2])
        wts = np.concatenate([w1[r1], w2[r2]]).astype(np.float32)
        per_expert.append((rows, wts))
    return per_expert


def _silu(v):
    return v / (1.0 + np.exp(-v))


def _sample_ok(out_flat, xf, per_expert, moe_w13, moe_w2):
    """Exact-fp32 spot check of ~128 tokens against the inputs; catches any
    widespread device-side corruption (observed failure mode: absmax ~18x)."""
    sample = np.arange(0, T, 64)
    ref = np.zeros((len(sample), D), dtype=np.float32)
    pos_of = {t: i for i, t in enumerate(sample)}
    sset = set(sample.tolist())
    for e in range(E):
        rows, wts = per_expert[e]
        sel = [i for i, t in enumerate(rows) if t in sset]
        if not sel:
            continue
        toks = rows[sel]
        xg = xf[toks].astype(np.float32)
        h13 = xg @ moe_w13[e].astype(np.float32)
        hid = _silu(h13[:, :M]) * h13[:, M:]
        o = hid @ moe_w2[e].astype(np.float32)
        for k, t in enumerate(toks):
            ref[pos_of[t]] += o[k] * wts[sel[k]]
    err = np.abs(out_flat[sample] - ref).max()
    return err <= 1.5e-2 * max(np.abs(ref).max(), 1e-6)


def kernel(x, moe_router, moe_w13, moe_w2, _trace=False, _trace_kwargs=None):
    global last_results
    x = np.asarray(x)
    moe_router = np.asarray(moe_router)
    moe_w13 = np.asarray(moe_w13)
    moe_w2 = np.asarray(moe_w2)
    xf = np.ascontiguousarray(x.reshape(T, D).astype(np.float32))
    per_expert = _route(xf, np.asarray(moe_router, dtype=np.float32))

    loads = [len(rows) for rows, _ in per_expert]
    order = sorted(range(E), key=lambda e: -loads[e])
    pairs = [(order[2 * i], order[2 * i + 1]) for i in range(NP)]
    Cs = []
    for ea, eb in pairs:
        C = max(loads[ea], loads[eb])
        Cs.append(max(C + (C & 1), 2 * TB))
    Cs = tuple(Cs)

    nc = _NC_CACHE.get(Cs)
    if nc is None:
        nc = _build(Cs)
        _NC_CACHE[Cs] = nc

    xf_bf = xf.astype(ml_dtypes.bfloat16)

    def staged_tokens(e, C):
        rows, _ = per_expert[e]
        xg = np.zeros((C, D), dtype=ml_dtypes.bfloat16)
        xg[: len(rows)] = xf_bf[rows]
        return _stage_x(xg)

    xt_by = {}
    for i, (ea, eb) in enumerate(pairs):
        xt_by[ea] = staged_tokens(ea, Cs[i])
        xt_by[eb] = staged_tokens(eb, Cs[i])

    # core c: phase i -> expert pairs[i][c // 4], hidden slice c % 4
    in_maps, slot_of = [], []
    for c in range(E):
        s = c % 4
        m = {}
        slots = []
        for i in range(NP):
            e = pairs[i][c // 4]
            w13e = np.asarray(moe_w13[e]).astype(ml_dtypes.bfloat16)
            m[f"xt{i}"] = xt_by[e]
            m[f"w13{i}"] = _stage_w13_slice(w13e, s, chunked=(i == 0))
            m[f"w2{i}"] = _stage_w2_slice(
                np.asarray(moe_w2[e]).astype(ml_dtypes.bfloat16), s
            )
            slots.append(e)
        in_maps.append(m)
        slot_of.append(slots)

    for attempt in range(3):
        res = run_bass_kernel_spmd(
            nc,
            in_maps,
            core_ids=list(range(E)),
            trace=_trace,
            **(_trace_kwargs or {}),
        )
        last_results = res

        out = np.zeros((T, D), dtype=np.float32)
        for core in range(E):
            for i in range(NP):
                e = slot_of[core][i]
                rows, wts = per_expert[e]
                ote = _unstage_o(np.asarray(res.results[core][f"ot{i}"]), Cs[i])
                out[rows] += ote[:, : len(rows)].T.astype(np.float32) * wts[:, None]
        if _sample_ok(out, xf, per_expert, moe_w13, moe_w2):
            break
        print(f"kernel: sample validation failed (attempt {attempt}), re-running")
    return out.reshape(B, S, D)


# revision 50
# speedup vs baseline: 1.0082x; 1.0082x over previous
"""Trainium2 Bass kernel: 8-expert top-2 MoE MLP (SwiGLU), 8 cores.

Hidden-dim-sliced expert parallelism, 4 phases: every expert's FFN is split
into four M/4=512 hidden slices (gate/up pairs stay together, so the split is
exact; the four mm2 partial outputs are summed on the host in the combine).
Experts are paired by adjacent load rank; phase i runs both experts of pair i
— cores 0-3 hold slices 0-3 of the first expert, cores 4-7 of the second —
with capacity C_i = max(pair_i loads).  Per-core work is sum(C_i)/4 ~ 2057
full-width token-equivalents vs 2150 under plain EP and 2079 under the
2-phase variant.  Weights per core stay 12.6 MB bf16.

Per phase, per core (one expert quarter-slice, MH4=512, KH4=4):
  H'^T[2*MH4, C] = (W13 slice stationary).T x X^T moving  (contract D)
  H^T = silu(gate) * up                                   (ACT + DVE)
  O^T[D, C]      = (W2 slice stationary) x H^T moving     (contract MH4)

x tiles share block tags across phases (bufs=2) so only ~2 phases of x are
SBUF-resident; phase i+2's x DMAs are emitted after phase i's compute so
their WAR waits are satisfied when the sync queue reaches them.

Schedule facts (trace-verified on ancestors): ~8.3us framework preamble;
8 warm-up matmuls fix the 1.2->2.4 GHz HAM throttle inside the DMA ramp;
matmuls issue at N/2.4+2.5 ns with LDWEIGHTS hidden; phase-0 block 0 is
chunk-gated k-granularly against the ~0.33 MB/us HBM stream; the kernel ends
on a 128-token block to shorten the final cast+DMA+teardown chain.  A host
spot-check of 128 tokens guards against the (rare, timing-dependent) device
corruption seen once on this rig; on mismatch the NEFF re-runs (<=2 retries).
"""

from contextlib import ExitStack

import ml_dtypes
import numpy as np

import concourse.bass as bass  # noqa: F401  (AP helpers)
import concourse.tile as tile
from concourse import bacc, mybir
from concourse.bass_utils import run_bass_kernel_spmd

# nn_MoEMLP_82617990905863 (hardcoded per contract)
B, S, D = 4, 2048, 1024
T = B * S               # 8192 tokens
E = 8                   # experts
TOPK = 2
M = 2048                # full MOE_DIM (w13 = [D, 2M], w2 = [M, D])
NP = 4                  # phases
MH4 = M // NP           # hidden slice width per core-phase = 512
TB = 512                # token block = moving free dim
KD = D // 128           # 8 contraction tiles for X @ W13
KH4 = MH4 // 128        # 4 contraction tiles for H @ W2 (per slice)
# phase-0 w13 chunk schedule over its 4 j-tiles: singles first
W13_CHUNKS = [[0], [1], [2, 3]]

_NC_CACHE: dict[tuple, object] = {}
last_results = None     # BassKernelResults of the most recent run (for test.py)


def _blocks(C: int, tail_split: bool = False) -> list[int]:
    sizes = []
    left = C
    while left > TB:
        sizes.append(TB)
        left -= TB
    if left:
        sizes.append(left)
    # tiny remainders run at the per-MM issue floor — merge them into the
    # previous block, then split so the kernel ends on a narrow block
    # (the final cast+DMA+teardown chain scales with last-block width)
    if len(sizes) >= 2 and sizes[-1] < 192:
        sizes[-2:] = [sizes[-2] + sizes[-1]]
    if tail_split and sizes[-1] > 192:
        cut = 256 if sizes[-1] > 640 else 128
        sizes[-1:] = [sizes[-1] - cut, cut]
    return sizes


def _build(Cs: tuple):
    """NP sequential expert-slice phases. Phase 0 has the chunk-gated DMA
    ramp; later phases' weights are fully prefetched."""
    dt = mybir.dt
    odt = dt.bfloat16
    nc = bacc.Bacc(
        "TRN2", target_bir_lowering=False, debug=False, enable_asserts=False
    )

    xt, w13, w2, ot = [], [], [], []
    for i, C in enumerate(Cs):
        xt.append(
            nc.dram_tensor(f"xt{i}", [128, KD * C], dt.bfloat16,
                           kind="ExternalInput").ap()
        )
        w13.append(
            nc.dram_tensor(f"w13{i}", [128, KD * 2 * MH4], dt.bfloat16,
                           kind="ExternalInput").ap()
        )
        w2.append(
            nc.dram_tensor(f"w2{i}", [128, KH4 * D], dt.bfloat16,
                           kind="ExternalInput").ap()
        )
        ot.append(
            nc.dram_tensor(f"ot{i}", [128, KD * C], odt,
                           kind="ExternalOutput").ap()
        )

    sizes = [_blocks(C, True) for C in Cs]
    offs = []
    for i in range(NP):
        o = [0]
        for s in sizes[i]:
            o.append(o[-1] + s)
        offs.append(o)

    with tile.TileContext(nc) as tc, ExitStack() as ctx:
        wpool = ctx.enter_context(tc.tile_pool(name="w", bufs=1))
        xpool = ctx.enter_context(tc.tile_pool(name="x", bufs=1))
        xring = ctx.enter_context(tc.tile_pool(name="xr", bufs=2))
        spool = ctx.enter_context(tc.tile_pool(name="s", bufs=4))
        hpool = ctx.enter_context(tc.tile_pool(name="h", bufs=2))
        opool = ctx.enter_context(tc.tile_pool(name="o", bufs=2))
        pg = ctx.enter_context(tc.tile_pool(name="pg", bufs=2, space="PSUM"))
        pu = ctx.enter_context(tc.tile_pool(name="pu", bufs=2, space="PSUM"))
        po = ctx.enter_context(tc.tile_pool(name="po", bufs=4, space="PSUM"))

        # --- PE clock warm-up during the DMA-bound ramp ---
        warm = xpool.tile([128, 512], dt.bfloat16, tag="warm")
        nc.vector.memset(warm[:], 0)
        wpsum = po.tile([128, 512], dt.float32, tag="po")
        for _ in range(8):
            nc.tensor.matmul(
                wpsum[:], warm[:, 0:128], warm[:, 0:512], start=True, stop=True
            )

        # --- phase-0 ramp: x block 0 and w13[0] chunk 0 as interleaved
        # k-pair tiles; w13[0] chunk 1 issued mid-stream ---
        x0k, w0q = [], []
        for q in range(4):
            tx = xpool.tile([128, 2 * TB], dt.bfloat16, tag=f"x0q{q}")
            nc.sync.dma_start(tx[:], xt[0][:, q * 2 * TB : (q + 1) * 2 * TB])
            x0k.append(tx)
            tw = wpool.tile([128, 512], dt.bfloat16, tag=f"w0q{q}")
            nc.sync.dma_start(tw[:], w13[0][:, q * 512 : (q + 1) * 512])
            w0q.append(tw)
            if q == 1:
                wc1 = wpool.tile([128, KD * 2 * 128], dt.bfloat16, tag="wc1")
                nc.sync.dma_start(
                    wc1[:], w13[0][:, KD * 256 : KD * 256 + KD * 2 * 128]
                )

        j_chunk = {}            # j -> (chunk idx, local jj, cgw)
        w13_offs = []
        off = 0
        for ci, js in enumerate(W13_CHUNKS):
            w13_offs.append(off)
            for jj, j in enumerate(js):
                j_chunk[j] = (ci, jj, 128 * len(js))
            off += KD * 2 * 128 * len(js)

        wt0 = [None, wc1]
        for ci in range(2, len(W13_CHUNKS)):
            cgw = 128 * len(W13_CHUNKS[ci])
            t = wpool.tile([128, KD * 2 * cgw], dt.bfloat16, tag=f"wc0{ci}")
            nc.sync.dma_start(
                t[:], w13[0][:, w13_offs[ci] : w13_offs[ci] + KD * 2 * cgw]
            )
            wt0.append(t)

        # x blocks share tags across phases via a bufs=2 ring; phase i+2's
        # loads are emitted after phase i's compute (WAR then satisfied by
        # the time the sync queue reaches them — no head-of-line jam)
        xb = [[None] * len(sizes[i]) for i in range(NP)]

        def load_x(i, b):
            n = sizes[i][b]
            t = xring.tile([128, KD * n], dt.bfloat16, tag=f"xb{b}", name="xb")
            xb[i][b] = t
            nc.sync.dma_start(
                t[:], xt[i][:, KD * offs[i][b] : KD * offs[i][b] + KD * n]
            )

        for b in range(1, len(sizes[0])):
            load_x(0, b)
        w2t, wtf = [None] * NP, [None] * NP
        w2t[0] = wpool.tile([128, KH4 * D], dt.bfloat16, tag="w2p0", name="w2t0")
        nc.sync.dma_start(w2t[0][:], w2[0][:, :])
        for i in range(1, NP):
            wtf[i] = wpool.tile([128, KD * 2 * MH4], dt.bfloat16, tag=f"wp{i}", name="wtf")
            nc.sync.dma_start(wtf[i][:], w13[i][:, :])
            w2t[i] = wpool.tile([128, KH4 * D], dt.bfloat16, tag=f"w2p{i}", name="w2t")
            nc.sync.dma_start(w2t[i][:], w2[i][:, :])
        for b in range(len(sizes[1])):
            load_x(1, b)

        def x_slice(i, b, k, n):
            if i == 0 and b == 0:
                q, kk = divmod(k, 2)
                return x0k[q][:, kk * n : (kk + 1) * n]
            return xb[i][b][:, k * n : (k + 1) * n]

        def w13_slice(i, j, k, gate):
            if i == 0:
                ci, jj, cgw = j_chunk[j]
                if ci == 0:
                    q, kk = divmod(k, 2)
                    base = kk * 256 + (0 if gate else 128)
                    return w0q[q][:, base : base + 128]
                base = k * 2 * cgw + (0 if gate else cgw) + jj * 128
                return wt0[ci][:, base : base + 128]
            base = k * 2 * MH4 + (0 if gate else MH4) + j * 128
            return wtf[i][:, base : base + 128]

        def emit_phase(i):
            for b in range(len(sizes[i])):
                n = sizes[i][b]
                h_t = []
                for j in range(KH4):
                    g = pg.tile([128, TB], dt.float32, tag="pg", name="g")[:, 0:n]
                    u = pu.tile([128, TB], dt.float32, tag="pu", name="u")[:, 0:n]
                    for k in range(KD):
                        nc.tensor.matmul(
                            g[:],
                            w13_slice(i, j, k, True),
                            x_slice(i, b, k, n),
                            start=(k == 0),
                            stop=(k == KD - 1),
                        )
                    for k in range(KD):
                        nc.tensor.matmul(
                            u[:],
                            w13_slice(i, j, k, False),
                            x_slice(i, b, k, n),
                            start=(k == 0),
                            stop=(k == KD - 1),
                        )
                    gs = spool.tile(
                        [128, TB], dt.float32, tag="gs", name="gs"
                    )[:, 0:n]
                    nc.scalar.activation(
                        gs[:], g[:], mybir.ActivationFunctionType.Silu
                    )
                    h = hpool.tile(
                        [128, TB], dt.bfloat16, tag=f"h{j}", name="h"
                    )[:, 0:n]
                    nc.vector.tensor_mul(h[:], gs[:], u[:])
                    h_t.append(h)
                ob = opool.tile(
                    [128, (KD - 1) * TB], odt, tag="o", name="ob"
                )[:, 0 : (KD - 1) * n]
                o7 = opool.tile([128, TB], odt, tag="o7", name="o7")[:, 0:n]
                for d in range(KD):
                    p = po.tile([128, TB], dt.float32, tag="po", name="p")[:, 0:n]
                    for j in range(KH4):
                        nc.tensor.matmul(
                            p[:],
                            w2t[i][:, j * D + d * 128 : j * D + (d + 1) * 128],
                            h_t[j][:],
                            start=(j == 0),
                            stop=(j == KH4 - 1),
                        )
                    if d < KD - 1:
                        nc.vector.tensor_copy(ob[:, d * n : (d + 1) * n], p[:])
                    else:
                        nc.vector.tensor_copy(o7[:], p[:])
                    if d == KD - 2:
                        nc.sync.dma_start(
                            ot[i][
                                :,
                                KD * offs[i][b] : KD * offs[i][b] + (KD - 1) * n,
                            ],
                            ob[:],
                        )
                nc.sync.dma_start(
                    ot[i][
                        :,
                        KD * offs[i][b] + (KD - 1) * n : KD * offs[i][b] + KD * n,
                    ],
                    o7[:],
                )

        for i in range(NP):
            emit_phase(i)
            if i + 2 < NP:
                for b in range(len(sizes[i + 2])):
                    load_x(i + 2, b)

    nc.compile()
    return nc


def _stage_x(xg: np.ndarray) -> np.ndarray:
    """[C, D] gathered tokens -> [128, block-major (b, k, tok)] bf16."""
    C = xg.shape[0]
    a = np.ascontiguousarray(xg.T).reshape(KD, 128, C)       # [k, p, tok]
    blocks = []
    c0 = 0
    for n in _blocks(C, True):
        blocks.append(a[:, :, c0 : c0 + n].transpose(1, 0, 2).reshape(128, KD * n))
        c0 += n
    return np.ascontiguousarray(np.concatenate(blocks, axis=1))


def _stage_w13_slice(w: np.ndarray, s: int, chunked: bool) -> np.ndarray:
    """[D, 2M] gate|up, hidden slice s of width MH4 -> [128, ...] bf16.
    chunked=True lays out per W13_CHUNKS (phase 0); else k-major full."""
    g = w[:, s * MH4 : (s + 1) * MH4]
    u = w[:, M + s * MH4 : M + (s + 1) * MH4]
    if not chunked:
        a = np.concatenate([g, u], axis=1)                    # [D, 2*MH4]
        return np.ascontiguousarray(
            a.reshape(KD, 128, 2 * MH4).transpose(1, 0, 2).reshape(
                128, KD * 2 * MH4
            )
        )
    parts = []
    for js in W13_CHUNKS:
        cgw = 128 * len(js)
        cols_g = np.concatenate([g[:, j * 128 : (j + 1) * 128] for j in js], axis=1)
        cols_u = np.concatenate([u[:, j * 128 : (j + 1) * 128] for j in js], axis=1)
        a = np.concatenate([cols_g, cols_u], axis=1)         # [D, 2cgw]
        parts.append(
            a.reshape(KD, 128, 2 * cgw).transpose(1, 0, 2).reshape(128, KD * 2 * cgw)
        )
    return np.ascontiguousarray(np.concatenate(parts, axis=1))


def _stage_w2_slice(w: np.ndarray, s: int) -> np.ndarray:
    """[M, D], hidden slice s -> [128, (k, d)] bf16."""
    ws = w[s * MH4 : (s + 1) * MH4, :]
    return np.ascontiguousarray(
        ws.reshape(KH4, 128, D).transpose(1, 0, 2).reshape(128, KH4 * D)
    )


def _unstage_o(ote: np.ndarray, C: int) -> np.ndarray:
    """[128, block-major (b, d, tok)] -> [D, C] (inverse of the x staging)."""
    blocks = []
    c0 = 0
    for n in _blocks(C, True):
        blk = ote[:, KD * c0 : KD * c0 + KD * n].reshape(128, KD, n)
        blocks.append(blk.transpose(1, 0, 2).reshape(D, n))
        c0 += n
    return np.concatenate(blocks, axis=1)


def _route(xf: np.ndarray, moe_router: np.ndarray):
    """Top-2 routing on host. Returns per-expert (rows, weights)."""
    logits = xf @ moe_router                      # [T, E] f32
    top1 = np.argmax(logits, axis=1)
    tmp = logits.copy()
    tmp[np.arange(T), top1] = -np.inf
    top2 = np.argmax(tmp, axis=1)
    l1 = logits[np.arange(T), top1]
    l2 = logits[np.arange(T), top2]
    mx = np.maximum(l1, l2)
    e1 = np.exp(l1 - mx)
    e2 = np.exp(l2 - mx)
    s = e1 + e2
    w1 = (e1 / s).astype(np.float32)
    w2 = (e2 / s).astype(np.float32)
    per_expert = []
    for e in range(E):
        r1 = np.where(top1 == e)[0]
        r2 = np.where(top2 == e)[0]
        rows = np.concatenate([r1, r2])
        wts = np.concatenate([w1[r1], w2[r2]]).astype(np.float32)
        per_expert.append((rows, wts))
    return per_expert


def _silu(v):
    return v / (1.0 + np.exp(-v))


def _sample_ok(out_flat, xf, per_expert, moe_w13, moe_w2):
    """Exact-fp32 spot check of ~128 tokens against the inputs; catches any
    widespread device-side corruption (observed failure mode: absmax ~18x)."""
    sample = np.arange(0, T, 64)
    ref = np.zeros((len(sample), D), dtype=np.float32)
    pos_of = {t: i for i, t in enumerate(sample)}
    sset = set(sample.tolist())
    for e in range(E):
        rows, wts = per_expert[e]
        sel = [i for i, t in enumerate(rows) if t in sset]
        if not sel:
            continue
        toks = rows[sel]
        xg = xf[toks].astype(np.float32)
        h13 = xg @ moe_w13[e].astype(np.float32)
        hid = _silu(h13[:, :M]) * h13[:, M:]
        o = hid @ moe_w2[e].astype(np.float32)
        for k, t in enumerate(toks):
            ref[pos_of[t]] += o[k] * wts[sel[k]]
    err = np.abs(out_flat[sample] - ref).max()
    return err <= 1.5e-2 * max(np.abs(ref).max(), 1e-6)


def kernel(x, moe_router, moe_w13, moe_w2, _trace=False, _trace_kwargs=None):
    global last_results
    x = np.asarray(x)
    moe_router = np.asarray(moe_router)
    moe_w13 = np.asarray(moe_w13)
    moe_w2 = np.asarray(moe_w2)
    xf = np.ascontiguousarray(x.reshape(T, D).astype(np.float32))
    per_expert = _route(xf, np.asarray(moe_router, dtype=np.float32))

    loads = [len(rows) for rows, _ in per_expert]
    order = sorted(range(E), key=lambda e: -loads[e])
    pairs = [(order[2 * i], order[2 * i + 1]) for i in range(NP)]
    Cs = []
    for ea, eb in pairs:
        C = max(loads[ea], loads[eb])
        Cs.append(max(C + (C & 1), 2 * TB))
    Cs = tuple(Cs)

    nc = _NC_CACHE.get(Cs)
    if nc is None:
        nc = _build(Cs)
        _NC_CACHE[Cs] = nc

    xf_bf = xf.astype(ml_dtypes.bfloat16)

    def staged_tokens(e, C):
        rows, _ = per_expert[e]
        xg = np.zeros((C, D), dtype=ml_dtypes.bfloat16)
        xg[: len(rows)] = xf_bf[rows]
        return _stage_x(xg)

    xt_by = {}
    for i, (ea, eb) in enumerate(pairs):
        xt_by[ea] = staged_tokens(ea, Cs[i])
        xt_by[eb] = staged_tokens(eb, Cs[i])

    # core c: phase i -> expert pairs[i][c // 4], hidden slice c % 4
    in_maps, slot_of = [], []
    for c in range(E):
        s = c % 4
        m = {}
        slots = []
        for i in range(NP):
            e = pairs[i][c // 4]
            w13e = np.asarray(moe_w13[e]).astype(ml_dtypes.bfloat16)
            m[f"xt{i}"] = xt_by[e]
            m[f"w13{i}"] = _stage_w13_slice(w13e, s, chunked=(i == 0))
            m[f"w2{i}"] = _stage_w2_slice(
                np.asarray(moe_w2[e]).astype(ml_dtypes.bfloat16), s
            )
            slots.append(e)
        in_maps.append(m)
        slot_of.append(slots)

    for attempt in range(3):
        res = run_bass_kernel_spmd(
            nc,
            in_maps,
            core_ids=list(range(E)),
            trace=_trace,
            **(_trace_kwargs or {}),
        )
        last_results = res

        out = np.zeros((T, D), dtype=np.float32)
        for core in range(E):
            for i in range(NP):
                e = slot_of[core][i]
                rows, wts = per_expert[e]
                ote = _unstage_o(np.asarray(res.results[core][f"ot{i}"]), Cs[i])
                out[rows] += ote[:, : len(rows)].T.astype(np.float32) * wts[:, None]
        if _sample_ok(out, xf, per_expert, moe_w13, moe_w2):
            break
        print(f"kernel: sample validation failed (attempt {attempt}), re-running")
    return out.reshape(B, S, D)


# revision 51
# speedup vs baseline: 1.0130x; 1.0048x over previous
"""Trainium2 Bass kernel: 8-expert top-2 MoE MLP (SwiGLU), 8 cores.

Hidden-dim-sliced expert parallelism, 4 phases: every expert's FFN is split
into four M/4=512 hidden slices (gate/up pairs stay together, so the split is
exact; the four mm2 partial outputs are summed on the host in the combine).
Experts are paired by adjacent load rank; phase i runs both experts of pair i
— cores 0-3 hold slices 0-3 of the first expert, cores 4-7 of the second —
with capacity C_i = max(pair_i loads).  Per-core work is sum(C_i)/4 ~ 2057
full-width token-equivalents vs 2150 under plain EP and 2079 under the
2-phase variant.  Weights per core stay 12.6 MB bf16.

Per phase, per core (one expert quarter-slice, MH4=512, KH4=4):
  H'^T[2*MH4, C] = (W13 slice stationary).T x X^T moving  (contract D)
  H^T = silu(gate) * up                                   (ACT + DVE)
  O^T[D, C]      = (W2 slice stationary) x H^T moving     (contract MH4)

x tiles share block tags across phases (bufs=2) so only ~2 phases of x are
SBUF-resident; phase i+2's x DMAs are emitted after phase i's compute so
their WAR waits are satisfied when the sync queue reaches them.

Schedule facts (trace-verified on ancestors): ~8.3us framework preamble;
8 warm-up matmuls fix the 1.2->2.4 GHz HAM throttle inside the DMA ramp;
matmuls issue at N/2.4+2.5 ns with LDWEIGHTS hidden; phase-0 block 0 is
chunk-gated k-granularly against the ~0.33 MB/us HBM stream; the kernel ends
on a 128-token block to shorten the final cast+DMA+teardown chain.  A host
spot-check of 128 tokens guards against the (rare, timing-dependent) device
corruption seen once on this rig; on mismatch the NEFF re-runs (<=2 retries).
"""

from contextlib import ExitStack

import ml_dtypes
import numpy as np

import concourse.bass as bass  # noqa: F401  (AP helpers)
import concourse.tile as tile
from concourse import bacc, mybir
from concourse.bass_utils import run_bass_kernel_spmd

# nn_MoEMLP_82617990905863 (hardcoded per contract)
B, S, D = 4, 2048, 1024
T = B * S               # 8192 tokens
E = 8                   # experts
TOPK = 2
M = 2048                # full MOE_DIM (w13 = [D, 2M], w2 = [M, D])
NP = 4                  # phases
MH4 = M // NP           # hidden slice width per core-phase = 512
TB = 512                # token block = moving free dim
KD = D // 128           # 8 contraction tiles for X @ W13
KH4 = MH4 // 128        # 4 contraction tiles for H @ W2 (per slice)
# phase-0 w13 chunk schedule over its 4 j-tiles: singles first
W13_CHUNKS = [[0], [1], [2, 3]]

_NC_CACHE: dict[tuple, object] = {}
last_results = None     # BassKernelResults of the most recent run (for test.py)


def _blocks(C: int, tail_split: bool = False) -> list[int]:
    sizes = []
    left = C
    while left > TB:
        sizes.append(TB)
        left -= TB
    if left:
        sizes.append(left)
    # tiny remainders run at the per-MM issue floor — merge them into the
    # previous block, then split so the kernel ends on a narrow block
    # (the final cast+DMA+teardown chain scales with last-block width)
    if len(sizes) >= 2 and sizes[-1] < 192:
        sizes[-2:] = [sizes[-2] + sizes[-1]]
    if tail_split and sizes[-1] > 192:
        cut = 256 if sizes[-1] > 640 else 128
        sizes[-1:] = [sizes[-1] - cut, cut]
    return sizes


def _build(Cs: tuple):
    """NP sequential expert-slice phases. Phase 0 has the chunk-gated DMA
    ramp; later phases' weights are fully prefetched."""
    dt = mybir.dt
    odt = dt.bfloat16
    nc = bacc.Bacc(
        "TRN2", target_bir_lowering=False, debug=False, enable_asserts=False
    )

    xt, w13, w2, ot = [], [], [], []
    for i, C in enumerate(Cs):
        xt.append(
            nc.dram_tensor(f"xt{i}", [128, KD * C], dt.bfloat16,
                           kind="ExternalInput").ap()
        )
        w13.append(
            nc.dram_tensor(f"w13{i}", [128, KD * 2 * MH4], dt.bfloat16,
                           kind="ExternalInput").ap()
        )
        w2.append(
            nc.dram_tensor(f"w2{i}", [128, KH4 * D], dt.bfloat16,
                           kind="ExternalInput").ap()
        )
        ot.append(
            nc.dram_tensor(f"ot{i}", [128, KD * C], odt,
                           kind="ExternalOutput").ap()
        )

    sizes = [_blocks(C, True) for C in Cs]
    offs = []
    for i in range(NP):
        o = [0]
        for s in sizes[i]:
            o.append(o[-1] + s)
        offs.append(o)

    with tile.TileContext(nc) as tc, ExitStack() as ctx:
        wpool = ctx.enter_context(tc.tile_pool(name="w", bufs=1))
        xpool = ctx.enter_context(tc.tile_pool(name="x", bufs=1))
        xring = ctx.enter_context(tc.tile_pool(name="xr", bufs=2))
        spool = ctx.enter_context(tc.tile_pool(name="s", bufs=4))
        hpool = ctx.enter_context(tc.tile_pool(name="h", bufs=2))
        opool = ctx.enter_context(tc.tile_pool(name="o", bufs=2))
        pg = ctx.enter_context(tc.tile_pool(name="pg", bufs=2, space="PSUM"))
        pu = ctx.enter_context(tc.tile_pool(name="pu", bufs=2, space="PSUM"))
        po = ctx.enter_context(tc.tile_pool(name="po", bufs=4, space="PSUM"))

        # --- PE clock warm-up during the DMA-bound ramp ---
        warm = xpool.tile([128, 512], dt.bfloat16, tag="warm")
        nc.vector.memset(warm[:], 0)
        wpsum = po.tile([128, 512], dt.float32, tag="po")
        for _ in range(10):
            nc.tensor.matmul(
                wpsum[:], warm[:, 0:128], warm[:, 0:512], start=True, stop=True
            )

        # --- phase-0 ramp: x block 0 and w13[0] chunk 0 as interleaved
        # k-pair tiles; w13[0] chunk 1 issued mid-stream ---
        x0k, w0q = [], []
        for q in range(4):
            tx = xpool.tile([128, 2 * TB], dt.bfloat16, tag=f"x0q{q}")
            nc.sync.dma_start(tx[:], xt[0][:, q * 2 * TB : (q + 1) * 2 * TB])
            x0k.append(tx)
            tw = wpool.tile([128, 512], dt.bfloat16, tag=f"w0q{q}")
            nc.sync.dma_start(tw[:], w13[0][:, q * 512 : (q + 1) * 512])
            w0q.append(tw)
            if q == 2:
                wc1a = wpool.tile([128, 4 * 256], dt.bfloat16, tag="wc1a")
                nc.sync.dma_start(
                    wc1a[:], w13[0][:, KD * 256 : KD * 256 + 4 * 256]
                )
        wc1b = wpool.tile([128, 4 * 256], dt.bfloat16, tag="wc1b")
        nc.sync.dma_start(
            wc1b[:], w13[0][:, KD * 256 + 4 * 256 : KD * 256 + 8 * 256]
        )

        j_chunk = {}            # j -> (chunk idx, local jj, cgw)
        w13_offs = []
        off = 0
        for ci, js in enumerate(W13_CHUNKS):
            w13_offs.append(off)
            for jj, j in enumerate(js):
                j_chunk[j] = (ci, jj, 128 * len(js))
            off += KD * 2 * 128 * len(js)

        wt0 = [None, (wc1a, wc1b)]
        for ci in range(2, len(W13_CHUNKS)):
            cgw = 128 * len(W13_CHUNKS[ci])
            t = wpool.tile([128, KD * 2 * cgw], dt.bfloat16, tag=f"wc0{ci}")
            nc.sync.dma_start(
                t[:], w13[0][:, w13_offs[ci] : w13_offs[ci] + KD * 2 * cgw]
            )
            wt0.append(t)

        # x blocks share tags across phases via a bufs=2 ring; phase i+2's
        # loads are emitted after phase i's compute (WAR then satisfied by
        # the time the sync queue reaches them — no head-of-line jam)
        xb = [[None] * len(sizes[i]) for i in range(NP)]

        def load_x(i, b):
            n = sizes[i][b]
            t = xring.tile([128, KD * n], dt.bfloat16, tag=f"xb{b}", name="xb")
            xb[i][b] = t
            nc.sync.dma_start(
                t[:], xt[i][:, KD * offs[i][b] : KD * offs[i][b] + KD * n]
            )

        for b in range(1, len(sizes[0])):
            load_x(0, b)
        w2t, wtf = [None] * NP, [None] * NP
        w2t[0] = wpool.tile([128, KH4 * D], dt.bfloat16, tag="w2p0", name="w2t0")
        nc.sync.dma_start(w2t[0][:], w2[0][:, :])
        for i in range(1, NP):
            wtf[i] = wpool.tile([128, KD * 2 * MH4], dt.bfloat16, tag=f"wp{i}", name="wtf")
            nc.sync.dma_start(wtf[i][:], w13[i][:, :])
            w2t[i] = wpool.tile([128, KH4 * D], dt.bfloat16, tag=f"w2p{i}", name="w2t")
            nc.sync.dma_start(w2t[i][:], w2[i][:, :])
        for b in range(len(sizes[1])):
            load_x(1, b)

        def x_slice(i, b, k, n):
            if i == 0 and b == 0:
                q, kk = divmod(k, 2)
                return x0k[q][:, kk * n : (kk + 1) * n]
            return xb[i][b][:, k * n : (k + 1) * n]

        def w13_slice(i, j, k, gate):
            if i == 0:
                ci, jj, cgw = j_chunk[j]
                if ci == 0:
                    q, kk = divmod(k, 2)
                    base = kk * 256 + (0 if gate else 128)
                    return w0q[q][:, base : base + 128]
                if ci == 1:
                    half = wt0[1][k // 4]
                    base = (k % 4) * 256 + (0 if gate else 128)
                    return half[:, base : base + 128]
                base = k * 2 * cgw + (0 if gate else cgw) + jj * 128
                return wt0[ci][:, base : base + 128]
            base = k * 2 * MH4 + (0 if gate else MH4) + j * 128
            return wtf[i][:, base : base + 128]

        def emit_phase(i):
            for b in range(len(sizes[i])):
                n = sizes[i][b]
                h_t = []
                for j in range(KH4):
                    g = pg.tile([128, TB], dt.float32, tag="pg", name="g")[:, 0:n]
                    u = pu.tile([128, TB], dt.float32, tag="pu", name="u")[:, 0:n]
                    for k in range(KD):
                        nc.tensor.matmul(
                            g[:],
                            w13_slice(i, j, k, True),
                            x_slice(i, b, k, n),
                            start=(k == 0),
                            stop=(k == KD - 1),
                        )
                    for k in range(KD):
                        nc.tensor.matmul(
                            u[:],
                            w13_slice(i, j, k, False),
                            x_slice(i, b, k, n),
                            start=(k == 0),
                            stop=(k == KD - 1),
                        )
                    gs = spool.tile(
                        [128, TB], dt.float32, tag="gs", name="gs"
                    )[:, 0:n]
                    nc.scalar.activation(
                        gs[:], g[:], mybir.ActivationFunctionType.Silu
                    )
                    h = hpool.tile(
                        [128, TB], dt.bfloat16, tag=f"h{j}", name="h"
                    )[:, 0:n]
                    nc.vector.tensor_mul(h[:], gs[:], u[:])
                    h_t.append(h)
                ob = opool.tile(
                    [128, (KD - 1) * TB], odt, tag="o", name="ob"
                )[:, 0 : (KD - 1) * n]
                o7 = opool.tile([128, TB], odt, tag="o7", name="o7")[:, 0:n]
                for d in range(KD):
                    p = po.tile([128, TB], dt.float32, tag="po", name="p")[:, 0:n]
                    for j in range(KH4):
                        nc.tensor.matmul(
                            p[:],
                            w2t[i][:, j * D + d * 128 : j * D + (d + 1) * 128],
                            h_t[j][:],
                            start=(j == 0),
                            stop=(j == KH4 - 1),
                        )
                    if d < KD - 1:
                        nc.vector.tensor_copy(ob[:, d * n : (d + 1) * n], p[:])
                    else:
                        nc.vector.tensor_copy(o7[:], p[:])
                    if d == KD - 2:
                        nc.sync.dma_start(
                            ot[i][
                                :,
                                KD * offs[i][b] : KD * offs[i][b] + (KD - 1) * n,
                            ],
                            ob[:],
                        )
                nc.sync.dma_start(
                    ot[i][
                        :,
                        KD * offs[i][b] + (KD - 1) * n : KD * offs[i][b] + KD * n,
                    ],
                    o7[:],
                )

        for i in range(NP):
            emit_phase(i)
            if i + 2 < NP:
                for b in range(len(sizes[i + 2])):
                    load_x(i + 2, b)

    nc.compile()
    return nc


def _stage_x(xg: np.ndarray) -> np.ndarray:
    """[C, D] gathered tokens -> [128, block-major (b, k, tok)] bf16."""
    C = xg.shape[0]
    a = np.ascontiguousarray(xg.T).reshape(KD, 128, C)       # [k, p, tok]
    blocks = []
    c0 = 0
    for n in _blocks(C, True):
        blocks.append(a[:, :, c0 : c0 + n].transpose(1, 0, 2).reshape(128, KD * n))
        c0 += n
    return np.ascontiguousarray(np.concatenate(blocks, axis=1))


def _stage_w13_slice(w: np.ndarray, s: int, chunked: bool) -> np.ndarray:
    """[D, 2M] gate|up, hidden slice s of width MH4 -> [128, ...] bf16.
    chunked=True lays out per W13_CHUNKS (phase 0); else k-major full."""
    g = w[:, s * MH4 : (s + 1) * MH4]
    u = w[:, M + s * MH4 : M + (s + 1) * MH4]
    if not chunked:
        a = np.concatenate([g, u], axis=1)                    # [D, 2*MH4]
        return np.ascontiguousarray(
            a.reshape(KD, 128, 2 * MH4).transpose(1, 0, 2).reshape(
                128, KD * 2 * MH4
            )
        )
    parts = []
    for js in W13_CHUNKS:
        cgw = 128 * len(js)
        cols_g = np.concatenate([g[:, j * 128 : (j + 1) * 128] for j in js], axis=1)
        cols_u = np.concatenate([u[:, j * 128 : (j + 1) * 128] for j in js], axis=1)
        a = np.concatenate([cols_g, cols_u], axis=1)         # [D, 2cgw]
        parts.append(
            a.reshape(KD, 128, 2 * cgw).transpose(1, 0, 2).reshape(128, KD * 2 * cgw)
        )
    return np.ascontiguousarray(np.concatenate(parts, axis=1))


def _stage_w2_slice(w: np.ndarray, s: int) -> np.ndarray:
    """[M, D], hidden slice s -> [128, (k, d)] bf16."""
    ws = w[s * MH4 : (s + 1) * MH4, :]
    return np.ascontiguousarray(
        ws.reshape(KH4, 128, D).transpose(1, 0, 2).reshape(128, KH4 * D)
    )


def _unstage_o(ote: np.ndarray, C: int) -> np.ndarray:
    """[128, block-major (b, d, tok)] -> [D, C] (inverse of the x staging)."""
    blocks = []
    c0 = 0
    for n in _blocks(C, True):
        blk = ote[:, KD * c0 : KD * c0 + KD * n].reshape(128, KD, n)
        blocks.append(blk.transpose(1, 0, 2).reshape(D, n))
        c0 += n
    return np.concatenate(blocks, axis=1)


def _route(xf: np.ndarray, moe_router: np.ndarray):
    """Top-2 routing on host. Returns per-expert (rows, weights)."""
    logits = xf @ moe_router                      # [T, E] f32
    top1 = np.argmax(logits, axis=1)
    tmp = logits.copy()
    tmp[np.arange(T), top1] = -np.inf
    top2 = np.argmax(tmp, axis=1)
    l1 = logits[np.arange(T), top1]
    l2 = logits[np.arange(T), top2]
    mx = np.maximum(l1, l2)
    e1 = np.exp(l1 - mx)
    e2 = np.exp(l2 - mx)
    s = e1 + e2
    w1 = (e1 / s).astype(np.float32)
    w2 = (e2 / s).astype(np.float32)
    per_expert = []
    for e in range(E):
        r1 = np.where(top1 == e)[0]
        r2 = np.where(top2 == e)[0]
        rows = np.concatenate([r1, r2])
        wts = np.concatenate([w1[r1], w2[r2]]).astype(np.float32)
        per_expert.append((rows, wts))
    return per_expert


def _silu(v):
    return v / (1.0 + np.exp(-v))


def _sample_ok(out_flat, xf, per_expert, moe_w13, moe_w2):
    """Exact-fp32 spot check of ~128 tokens against the inputs; catches any
    widespread device-side corruption (observed failure mode: absmax ~18x)."""
    sample = np.arange(0, T, 64)
    ref = np.zeros((len(sample), D), dtype=np.float32)
    pos_of = {t: i for i, t in enumerate(sample)}
    sset = set(sample.tolist())
    for e in range(E):
        rows, wts = per_expert[e]
        sel = [i for i, t in enumerate(rows) if t in sset]
        if not sel:
            continue
        toks = rows[sel]
        xg = xf[toks].astype(np.float32)
        h13 = xg @ moe_w13[e].astype(np.float32)
        hid = _silu(h13[:, :M]) * h13[:, M:]
        o = hid @ moe_w2[e].astype(np.float32)
        for k, t in enumerate(toks):
            ref[pos_of[t]] += o[k] * wts[sel[k]]
    err = np.abs(out_flat[sample] - ref).max()
    return err <= 1.5e-2 * max(np.abs(ref).max(), 1e-6)


def kernel(x, moe_router, moe_w13, moe_w2, _trace=False, _trace_kwargs=None):
    global last_results
    x = np.asarray(x)
    moe_router = np.asarray(moe_router)
    moe_w13 = np.asarray(moe_w13)
    moe_w2 = np.asarray(moe_w2)
    xf = np.ascontiguousarray(x.reshape(T, D).astype(np.float32))
    per_expert = _route(xf, np.asarray(moe_router, dtype=np.float32))

    loads = [len(rows) for rows, _ in per_expert]
    order = sorted(range(E), key=lambda e: -loads[e])
    pairs = [(order[2 * i], order[2 * i + 1]) for i in range(NP)]
    Cs = []
    for ea, eb in pairs:
        C = max(loads[ea], loads[eb])
        Cs.append(max(C + (C & 1), 2 * TB))
    Cs = tuple(Cs)

    nc = _NC_CACHE.get(Cs)
    if nc is None:
        nc = _build(Cs)
        _NC_CACHE[Cs] = nc

    xf_bf = xf.astype(ml_dtypes.bfloat16)

    def staged_tokens(e, C):
        rows, _ = per_expert[e]
        xg = np.zeros((C, D), dtype=ml_dtypes.bfloat16)
        xg[: len(rows)] = xf_bf[rows]
        return _stage_x(xg)

    xt_by = {}
    for i, (ea, eb) in enumerate(pairs):
        xt_by[ea] = staged_tokens(ea, Cs[i])
        xt_by[eb] = staged_tokens(eb, Cs[i])

    # core c: phase i -> expert pairs[i][c // 4], hidden slice c % 4
    in_maps, slot_of = [], []
    for c in range(E):
        s = c % 4
        m = {}
        slots = []
        for i in range(NP):
            e = pairs[i][c // 4]
            w13e = np.asarray(moe_w13[e]).astype(ml_dtypes.bfloat16)
            m[f"xt{i}"] = xt_by[e]
            m[f"w13{i}"] = _stage_w13_slice(w13e, s, chunked=(i == 0))
            m[f"w2{i}"] = _stage_w2_slice(
                np.asarray(moe_w2[e]).astype(ml_dtypes.bfloat16), s
            )
            slots.append(e)
        in_maps.append(m)
        slot_of.append(slots)

    for attempt in range(3):
        res = run_bass_kernel_spmd(
            nc,
            in_maps,
            core_ids=list(range(E)),
            trace=_trace,
            **(_trace_kwargs or {}),
        )
        last_results = res

        out = np.zeros((T, D), dtype=np.float32)
        for core in range(E):
            for i in range(NP):
                e = slot_of[core][i]
                rows, wts = per_expert[e]
                ote = _unstage_o(np.asarray(res.results[core][f"ot{i}"]), Cs[i])
                out[rows] += ote[:, : len(rows)].T.astype(np.float32) * wts[:, None]
        if _sample_ok(out, xf, per_expert, moe_w13, moe_w2):
            break
        print(f"kernel: sample validation failed (attempt {attempt}), re-running")
    return out.reshape(B, S, D)
